# revision 21
# baseline (speedup 1.0000x reference)
"""Trainium2 Bass kernel for an RNN-T decoder (2-layer LSTMCell + joint network).

Sharding: batch B=8 -> 8 cores (1 batch element per core). Each core runs its
own batch element's full pipeline: embedding gather, 2-layer LSTM (layer-
pipelined, U=80 steps), joint network (T*U x J @ J x V) and writes its
(T, U, V) slice of the output. No collectives needed.

Matmul convention: out[M,N] = lhsT[K,M].T @ rhs[K,N]; fp32 data is bitcast to
float32r for 1-cycle/row streaming where N >= 256.
"""

import os
import sys

for _p in ("/opt/trn_rl_repo",):
    if _p not in sys.path and os.path.isdir(_p):
        sys.path.insert(0, _p)

import numpy as np

import concourse.bass as bass
import concourse.mybir as mybir
from concourse.bass import AP
from concourse.bass_utils import run_bass_kernel_spmd
from concourse.masks import make_identity
from concourse.tile import TileContext

F32 = mybir.dt.float32
F32R = mybir.dt.float32r
I32 = mybir.dt.int32
AF = mybir.ActivationFunctionType

# Problem shapes (per core: one batch element)
B = 8
T = 160
U = 80
E = 512   # eprojs / encoder feature dim
D = 512   # LSTM hidden
EMB = 512
J = 512   # joint dim
V = 500   # vocab
G = 4 * D  # gates = 2048
KC = 4    # 512 = 4 chunks of 128
UBLK = 8  # u-block size for the joint
N_JOINT_ROWS = UBLK * T  # 1280 rows per joint block
N_CORES = 8


def r32(ap):
    return ap if ap.dtype == F32R else ap.bitcast(F32R)


def split_waits(nc):
    """Walrus codegen rejects >1 sync wait on 4-byte Matmult (LW struct) and
    Drain (CTRL_NO struct). Move extra waits onto preceding same-engine NoOps
    (waits execute in order before the instruction, so this is equivalent)."""
    NO_SPLIT = {"EventSemaphore"}
    for fn in nc.m.functions:
        for blk in fn.blocks:
            new = []
            for inst in blk.instructions:
                si = getattr(inst, "sync_info", None)
                if (getattr(inst, "opcode", None) not in NO_SPLIT and si is not None
                        and si.on_wait is not None and len(si.on_wait) > 1):
                    waits = list(si.on_wait)
                    for j, w in enumerate(waits[:-1]):
                        nop = mybir.InstNoOp(
                            name=f"{inst.name}-wsplit{j}", ins=[], outs=[])
                        nop.engine = inst.engine
                        nop.sync_info = mybir.SyncInfo(on_wait=[w], on_update=[])
                        new.append(nop)
                    inst.sync_info = mybir.SyncInfo(
                        on_wait=[waits[-1]], on_update=list(si.on_update or []))
                new.append(inst)
            blk.instructions[:] = new


def build_program():
    nc = bass.Bass()

    # ---- I/O ----
    hsT = nc.declare_dram_parameter("hsT", [E, T], F32R, isOutput=False)
    ys = nc.declare_dram_parameter("ys", [U], I32, isOutput=False)
    embed = nc.declare_dram_parameter("embed", [V, EMB], F32, isOutput=False)
    wih0T = nc.declare_dram_parameter("wih0T", [EMB, G], F32R, isOutput=False)
    whh0T = nc.declare_dram_parameter("whh0T", [D, G], F32R, isOutput=False)
    wih1T = nc.declare_dram_parameter("wih1T", [D, G], F32R, isOutput=False)
    whh1T = nc.declare_dram_parameter("whh1T", [D, G], F32R, isOutput=False)
    bias0 = nc.declare_dram_parameter("bias0", [1, G], F32R, isOutput=False)
    bias1 = nc.declare_dram_parameter("bias1", [1, G], F32R, isOutput=False)
    encwT = nc.declare_dram_parameter("encwT", [E, J], F32R, isOutput=False)
    encb = nc.declare_dram_parameter("encb", [1, J], F32R, isOutput=False)
    decwT = nc.declare_dram_parameter("decwT", [D, J], F32R, isOutput=False)
    outwT = nc.declare_dram_parameter("outwT", [J, V], F32R, isOutput=False)
    outb = nc.declare_dram_parameter("outb", [1, V], F32R, isOutput=False)
    ones_in = nc.declare_dram_parameter("ones_in", [1, 256], F32R, isOutput=False)
    out = nc.declare_dram_parameter("out", [T, U, V], F32, isOutput=True)

    with TileContext(nc) as tc:
        # ---------------- pools ----------------
        const_pool = tc.alloc_tile_pool(name="const", bufs=1)
        wpool = tc.alloc_tile_pool(name="weights", bufs=1)
        state_pool = tc.alloc_tile_pool(name="state", bufs=1)
        gpool = tc.alloc_tile_pool(name="gates_sb", bufs=2)
        lstm_psum = tc.alloc_tile_pool(name="lstm_ps", bufs=1, space="PSUM")
        htr_psum = tc.alloc_tile_pool(name="htr_ps", bufs=1, space="PSUM")

        ones = const_pool.tile([1, 256], F32R)
        nc.sync.dma_start(out=ones[:], in_=ones_in[:])
        ident = const_pool.tile([128, 128], F32)
        make_identity(nc, ident[:])

        # ---------------- load weights ----------------
        def load_chunked(name, dram_ap, rows, cols):
            tiles = []
            for k in range(rows // 128):
                tl = wpool.tile([128, cols], F32R, tag=f"{name}{k}")
                nc.sync.dma_start(out=tl[:], in_=dram_ap[k * 128:(k + 1) * 128, :])
                tiles.append(tl)
            return tiles

        wrec = tc.alloc_tile_pool(name="wrec", bufs=8)

        def load_rec(dram_ap):
            tiles = []
            for k in range(KC):
                tl = wrec.tile([128, G], F32R, tag="wrec")
                nc.sync.dma_start(out=tl[:], in_=dram_ap[k * 128:(k + 1) * 128, :])
                tiles.append(tl)
            return tiles

        wih0_t = load_rec(wih0T[:])
        whh0_t = load_rec(whh0T[:])
        encw_t = load_chunked("encw", encwT[:], E, J)
        decw_t = load_chunked("decw", decwT[:], D, J)
        outw_t = load_chunked("outw", outwT[:], J, V)

        bias1_sb = wpool.tile([1, G], F32R)
        nc.sync.dma_start(out=bias1_sb[:], in_=bias1[:])
        encb_sb = wpool.tile([1, J], F32R)
        nc.sync.dma_start(out=encb_sb[:], in_=encb[:])
        outb_sb = wpool.tile([1, V], F32R)
        nc.sync.dma_start(out=outb_sb[:], in_=outb[:])

        # hsT [E, T] -> [128, 4*T] (chunk c at cols [c*T:(c+1)*T])
        hsT_sb = wpool.tile([128, KC * T], F32R)
        nc.sync.dma_start(
            out=hsT_sb[:].rearrange("p (c t) -> p c t", c=KC),
            in_=hsT[:].rearrange("(c p) t -> p c t", p=128),
        )

        # ---------------- setup phase ----------------
        with tc.tile_pool(name="setup_ps", bufs=2, space="PSUM") as setup_ps, \
             tc.tile_pool(name="setup_sb", bufs=1) as setup_sb:
            # embedding gather: eys[u, :] = embed[ys[u], :]
            ys_sb = setup_sb.tile([U, 1], I32, tag="ys")
            nc.sync.dma_start(out=ys_sb[:], in_=ys[:].unsqueeze(1))
            eys_sb = setup_sb.tile([U, EMB], F32, tag="eys")
            nc.gpsimd.indirect_dma_start(
                out=eys_sb[:],
                out_offset=None,
                in_=embed[:],
                in_offset=bass.IndirectOffsetOnAxis(ap=ys_sb[:, :1], axis=0),
            )
            # eysT [128, KC*U]: transpose eys chunks on PE
            eysT_sb = wpool.tile([128, KC * U], F32R)
            for k in range(KC):
                ps = setup_ps.tile([128, U], F32, tag="sps", space="PSUM")
                nc.tensor.transpose(
                    out=ps[:], in_=eys_sb[:U, k * 128:(k + 1) * 128],
                    identity=ident[:U, :U],
                )
                nc.vector.tensor_copy(out=eysT_sb[:, k * U:(k + 1) * U], in_=ps[:])

            # encT [128, KC*T]: encT[j, t] = sum_e encwT[e, j] * hsT[e, t] + encb[j]
            encT_sb = wpool.tile([128, KC * T], F32)
            for j in range(KC):
                ps = setup_ps.tile([128, T], F32, tag="sps", space="PSUM")
                for k in range(KC):
                    nc.tensor.matmul(
                        out=ps[:],
                        lhsT=r32(encw_t[k][:, j * 128:(j + 1) * 128]),
                        rhs=r32(hsT_sb[:, k * T:(k + 1) * T]),
                        start=(k == 0), stop=False,
                    )
                nc.tensor.matmul(
                    out=ps[:],
                    lhsT=r32(encb_sb[:1, j * 128:(j + 1) * 128]),
                    rhs=r32(ones[:1, :T]),
                    start=False, stop=True,
                )
                nc.vector.tensor_copy(out=encT_sb[:, j * T:(j + 1) * T], in_=ps[:])

            # X0 [U, G] = eys @ wih0T + bias0 (both LSTM bias terms pre-summed)
            bias0_sb = setup_sb.tile([1, G], F32R, tag="bias0")
            nc.sync.dma_start(out=bias0_sb[:], in_=bias0[:])
            x0_dram = nc.dram_tensor("x0_scratch", [U, G], F32R)
            for s in range(4):
                ps = setup_ps.tile([U, 512], F32, tag="sps", space="PSUM")
                for k in range(KC):
                    nc.tensor.matmul(
                        out=ps[:],
                        lhsT=r32(eysT_sb[:, k * U:(k + 1) * U]),
                        rhs=r32(wih0_t[k][:, s * 512:(s + 1) * 512]),
                        start=(k == 0), stop=False,
                    )
                nc.tensor.matmul(
                    out=ps[:],
                    lhsT=r32(ones[:1, :U]),
                    rhs=r32(bias0_sb[:1, s * 512:(s + 1) * 512]),
                    start=False, stop=True,
                )
                xstg = setup_sb.tile([U, 512], F32R, tag="xstg")
                nc.vector.tensor_copy(out=xstg[:], in_=ps[:])
                nc.sync.dma_start(out=x0_dram[:, s * 512:(s + 1) * 512], in_=xstg[:])

        # ---------------- LSTM ----------------
        wih1_t = load_rec(wih1T[:])
        h0T = state_pool.tile([128, KC * U], F32R)   # h0T[d%?, k*U+t] layout: chunk k col t
        h1T = state_pool.tile([128, KC * U], F32R)
        c0_st = state_pool.tile([1, D], F32, tag="c0")
        c1_st = state_pool.tile([1, D], F32, tag="c1")
        c_st = [c0_st, c1_st]
        nc.vector.memset(c_st[0][:], 0.0)
        nc.vector.memset(c_st[1][:], 0.0)

        def lstm_step(t, layer, w_t, x_sb, hT_prev_seq, hT_out_seq, c_tile):
            """One LSTM cell step. The x-part (precomputed, incl. biases) is
            staged to partition 0 by DMA and folded in as a K=1 matmul."""
            xr = gpool.tile([1, G], F32R, tag="xr")
            nc.sync.dma_start(out=xr[:], in_=x_sb[t:t + 1, :])
            # gate layout (host-reordered): i | f | o | g
            ps_ifo = lstm_psum.tile([1, 1536], F32, tag="gifo", space="PSUM")
            ps_g = lstm_psum.tile([1, 512], F32, tag="gg", space="PSUM")
            for s in range(4):
                dst = ps_ifo[:1, s * 512:(s + 1) * 512] if s < 3 else ps_g[:]
                if t > 0:
                    for k in range(KC):
                        nc.tensor.matmul(
                            out=dst,
                            lhsT=r32(hT_prev_seq[:, k * U + (t - 1):k * U + t]),
                            rhs=r32(w_t[k][:, s * 512:(s + 1) * 512]),
                            start=(k == 0), stop=False,
                        )
                nc.tensor.matmul(
                    out=dst,
                    lhsT=r32(ones[:1, :1]),
                    rhs=r32(xr[:1, s * 512:(s + 1) * 512]),
                    start=(t == 0), stop=True,
                )
            gb = gpool.tile([1, G], F32, tag="gb")
            nc.scalar.activation(gb[:1, 0:1536], ps_ifo[:], AF.Sigmoid)
            nc.scalar.activation(gb[:1, 1536:2048], ps_g[:], AF.Tanh)
            gi = gb[:1, 0:512]
            gf = gb[:1, 512:1024]
            go = gb[:1, 1024:1536]
            gg = gb[:1, 1536:2048]
            t1 = gpool.tile([1, D], F32, tag="t1")
            t2 = gpool.tile([1, D], F32, tag="t2")
            nc.vector.tensor_tensor(
                out=t1[:], in0=gf, in1=c_tile[:], op=mybir.AluOpType.mult,
            )
            nc.gpsimd.tensor_tensor(
                out=t2[:], in0=gi, in1=gg, op=mybir.AluOpType.mult,
            )
            nc.vector.tensor_tensor(
                out=c_tile[:], in0=t1[:], in1=t2[:], op=mybir.AluOpType.add,
            )
            tcb = gpool.tile([1, D], F32, tag="tcb")
            nc.scalar.activation(tcb[:], c_tile[:], AF.Tanh)
            hb = gpool.tile([1, D], F32, tag="hb")
            nc.vector.tensor_tensor(
                out=hb[:], in0=go, in1=tcb[:], op=mybir.AluOpType.mult,
            )
            # transpose h [1, 512] -> hT chunks [128, 1] via PE
            ptr = htr_psum.tile([128, KC], F32, tag="htr", space="PSUM")
            for k in range(KC):
                nc.tensor.transpose(
                    out=ptr[:, k:k + 1],
                    in_=hb[:1, k * 128:(k + 1) * 128],
                    identity=ident[:1, :1],
                )
            # scatter the 4 chunks into hT_out_seq columns {k*U + t}
            nc.vector.tensor_copy(
                out=hT_out_seq[:].rearrange("p (k u) -> p u k", u=U)[:, t, :],
                in_=ptr[:],
            )

        # ---- layer 0, all steps ----
        for t in range(U):
            lstm_step(t, 0, whh0_t, x0_dram[:], h0T[:], h0T[:], c_st[0][:])

        whh1_t = load_rec(whh1T[:])

        # ---- X1 [U, G] = h0_seq @ wih1T + bias1 ----
        x1_dram = nc.dram_tensor("x1_scratch", [U, G], F32R)
        with tc.tile_pool(name="x1_ps", bufs=2, space="PSUM") as x1_ps:
            for s in range(4):
                ps = x1_ps.tile([U, 512], F32, tag="sps", space="PSUM")
                for k in range(KC):
                    nc.tensor.matmul(
                        out=ps[:],
                        lhsT=r32(h0T[:, k * U:(k + 1) * U]),
                        rhs=r32(wih1_t[k][:, s * 512:(s + 1) * 512]),
                        start=(k == 0), stop=False,
                    )
                nc.tensor.matmul(
                    out=ps[:],
                    lhsT=r32(ones[:1, :U]),
                    rhs=r32(bias1_sb[:1, s * 512:(s + 1) * 512]),
                    start=False, stop=True,
                )
                x1stg = gpool.tile([U, 512], F32R, tag="xstg")
                nc.vector.tensor_copy(out=x1stg[:], in_=ps[:])
                nc.sync.dma_start(out=x1_dram[:, s * 512:(s + 1) * 512], in_=x1stg[:])

        # ---- layer 1 + joint blocks ----
        decT_sb = state_pool.tile([128, KC * U], F32)
        joint_ps = tc.alloc_tile_pool(name="joint_ps", bufs=2, space="PSUM")
        zpool = tc.alloc_tile_pool(name="zpre", bufs=1)

        def joint_block(mb):
            u0 = mb * UBLK
            # dec projection for u in [u0, u0+UBLK): decT[j, u] = sum_d decwT[d, j] h1T[d, u]
            for j in range(KC):
                ps = joint_ps.tile([128, UBLK], F32, tag="jt", space="PSUM")
                for k in range(KC):
                    nc.tensor.matmul(
                        out=ps[:],
                        lhsT=r32(decw_t[k][:, j * 128:(j + 1) * 128]),
                        rhs=r32(h1T[:, k * U + u0:k * U + u0 + UBLK]),
                        start=(k == 0), stop=(k == KC - 1),
                    )
                nc.vector.tensor_copy(
                    out=decT_sb[:, j * U + u0:j * U + u0 + UBLK], in_=ps[:],
                )
            # zpre[c][:, (u-u0)*T + t] = tanh(decT[c][:, u] + encT[c][:, t])
            zts = []
            for c in range(KC):
                zt = zpool.tile([128, N_JOINT_ROWS], F32R, tag=f"z{c}")
                zts.append(zt)
                dec_b = decT_sb[:, c * U + u0:c * U + u0 + UBLK] \
                    .unsqueeze(2).broadcast_to([128, UBLK, T])
                enc_b = encT_sb[:, c * T:(c + 1) * T] \
                    .unsqueeze(1).broadcast_to([128, UBLK, T])
                nc.vector.tensor_tensor(
                    out=zt[:].rearrange("p (u t) -> p u t", u=UBLK),
                    in0=dec_b, in1=enc_b, op=mybir.AluOpType.add,
                )
                nc.scalar.activation(zt[:], zt[:], AF.Tanh)
            # joint matmul: rows r = (u-u0)*T + t; out[r, v] += zT[j, r] * outwT[j, v]
            for m in range(N_JOINT_ROWS // 128):
                ps = joint_ps.tile([128, V], F32, tag="jt", space="PSUM")
                for k in range(KC):
                    nc.tensor.matmul(
                        out=ps[:],
                        lhsT=r32(zts[k][:, m * 128:(m + 1) * 128]),
                        rhs=r32(outw_t[k][:]),
                        start=(k == 0), stop=False,
                    )
                nc.tensor.matmul(
                    out=ps[:], lhsT=r32(ones[:1, :128]), rhs=r32(outb_sb[:1, :]),
                    start=False, stop=True,
                )
                jout = zpool.tile([128, V], F32, tag="jout")
                if m % 2 == 0:
                    nc.scalar.copy(out=jout[:], in_=ps[:])
                else:
                    nc.vector.tensor_copy(out=jout[:], in_=ps[:])
                # DMA rows out: r_global = u*T + t, dram offset = t*(U*V) + u*V
                r0 = u0 * T + m * 128
                left = 128
                while left > 0:
                    u = r0 // T
                    t0 = r0 % T
                    n = min(left, T - t0)
                    nc.sync.dma_start(
                        out=out[t0:t0 + n, u, :],
                        in_=jout[128 - left:128 - left + n, :],
                    )
                    r0 += n
                    left -= n

        for t in range(U):
            lstm_step(t, 1, whh1_t, x1_dram[:], h1T[:], h1T[:], c_st[1][:])
            if t % UBLK == UBLK - 1:
                joint_block(t // UBLK)

        for _pool in (zpool, joint_ps, wrec, htr_psum, lstm_psum, gpool,
                      state_pool, wpool, const_pool):
            _pool.release()

    return nc


_NC_CACHE = None


def _get_nc():
    global _NC_CACHE
    if _NC_CACHE is None:
        _NC_CACHE = build_program()
        split_waits(_NC_CACHE)  # CoreSim can't handle the NoOps; HW needs them
    return _NC_CACHE


def kernel(hs_pad, ys_in_pad, embed, W_ih0, W_hh0, b_ih0, b_hh0,
           W_ih1, W_hh1, b_ih1, b_hh1,
           lin_enc_w, lin_enc_b, lin_dec_w, lin_out_w, lin_out_b):
    hs_pad = np.ascontiguousarray(np.asarray(hs_pad, dtype=np.float32))
    ys = np.asarray(ys_in_pad).astype(np.int32)
    f = lambda x: np.ascontiguousarray(np.asarray(x, dtype=np.float32))

    # device gate order: i | f | o | g  (torch order is i, f, g, o)
    gperm = np.r_[0:1024, 1536:2048, 1024:1536]
    shared = {
        "embed": f(embed),
        "wih0T": f(np.asarray(W_ih0).T[:, gperm]),
        "whh0T": f(np.asarray(W_hh0).T[:, gperm]),
        "wih1T": f(np.asarray(W_ih1).T[:, gperm]),
        "whh1T": f(np.asarray(W_hh1).T[:, gperm]),
        "bias0": f((np.asarray(b_ih0) + np.asarray(b_hh0))[gperm]).reshape(1, G),
        "bias1": f((np.asarray(b_ih1) + np.asarray(b_hh1))[gperm]).reshape(1, G),
        "encwT": f(np.asarray(lin_enc_w).T),
        "encb": f(lin_enc_b).reshape(1, J),
        "decwT": f(np.asarray(lin_dec_w).T),
        "outwT": f(np.asarray(lin_out_w).T),
        "outb": f(lin_out_b).reshape(1, V),
        "ones_in": np.ones((1, 256), np.float32),
    }
    in_maps = []
    for b in range(N_CORES):
        m = dict(shared)
        m["hsT"] = f(hs_pad[b].T)          # [E, T]
        m["ys"] = np.ascontiguousarray(ys[b])  # [U]
        in_maps.append(m)

    nc = _get_nc()
    res = run_bass_kernel_spmd(nc, in_maps, list(range(N_CORES)))
    outs = [np.asarray(res.results[b]["out"]) for b in range(N_CORES)]
    return np.stack(outs, axis=0)


if __name__ == "__main__":
    nc = build_program()
    print("program built ok;",
          sum(len(f.instructions) if hasattr(f, 'instructions') else 0
              for f in nc.m.functions), "instructions")
